# revision 13
# baseline (speedup 1.0000x reference)
"""Trainium2 Bass kernel for ConditionedPNA (3-layer PNAConv, N=50000, D=128, DEG=12).

Sharding/strategy (8 NeuronCores, SPMD):
  - Nodes sharded N/8 per core (padded to a multiple of 128). edge_index row is
    sorted (repeat(arange(N), DEG)), so each node's DEG edges are colocated with
    the node's core; segment reductions are purely local after an AllGather of
    the per-node message term B.
  - Algebra: m_e = C[row_e] + B[col_e] with B = h @ W2.T, C = h @ W1.T + bpre.
    All aggregators reduce to segment stats of B[col]:
      mean = C + S/12, max = C + MX, min = C + MN,
      std = sqrt(relu(S2/12 - (S/12)^2) + 1e-5)           (C cancels in var).
    deg == DEG everywhere -> degree scalers are constants folded into the
    post weights on the host. The C-dependent terms are LINEAR in h, so they
    fold into the h-GEMM: Whh = Wlin@Wh + (Wm+Wx+Wn)@W1; the bpre part goes
    to the bias. Residual is applied on-device in f32.
  - Data path is bf16: the allgathered table rows are [B | B^2] (512B rows),
    so S2 needs no on-chip square. Per 128-node tile: 12 indirect DMAs
    (one per edge slot, 128 rows each, 4 SWDGE queues) gather the table rows;
    DVE reduction trees (bf16, f32 final level) produce S/MX/MN/S2; the
    var chain runs on the Pool engine; sqrt(var/144+eps) is one ActE op;
    PE transposes the four parts (f32) and accumulates the post-GEMM
    hp = Whh@h^T + (Wm/12)@S^T + Wx@MX^T + Wn@MN^T + Ws@std^T in PSUM (bf16
    matmuls, f32 accum); hout = hp + bias + hin (f32).
  - B-production for layer l+1 is fused into layer l's tile loop, and the
    AllGather is chunked (Q chunks) and fired as chunks complete, so the
    collective overlaps the remaining tile-loop compute.
"""
import math
import numpy as np
import ml_dtypes

import concourse.bass as bass
from concourse import bacc
import concourse.tile as tile
from concourse import mybir
from concourse.masks import make_identity
from concourse.bass_utils import run_bass_kernel_spmd

D, DEG, L, CORES = 128, 12, 3, 8
NQ = 4   # SWDGE queues
Q = 4    # collective chunks
F32 = mybir.dt.float32
BF16 = mybir.dt.bfloat16
I32 = mybir.dt.int32
BF = ml_dtypes.bfloat16

_hist = np.array([1.0] * 10 + [2.0] * 10)
AVG_LOG = float((np.log(np.arange(20) + 1.0) * _hist).sum() / _hist.sum())


class Cfg:
    def __init__(self, n):
        self.N = n
        self.NC = n // CORES
        self.NP = ((self.NC + 127) // 128) * 128
        self.TILES = self.NP // 128
        self.AG_ROWS = CORES * self.NP
        # collective chunk boundaries, in tiles
        b = [round(q * self.TILES / Q) for q in range(Q + 1)]
        self.bounds = b
        self.chunk_of_tile = np.searchsorted(np.array(b[1:]), np.arange(self.TILES), side="right")


def _tree(nc, gm_half, work, out, opname):
    """Reduce the DEG axis of gm_half (128, 12, D) bf16; out (128, D) f32."""
    A = mybir.AluOpType
    op = {"add": A.add, "max": A.max, "min": A.min}[opname]
    v = nc.vector
    v.tensor_tensor(out=work[:, 0:6, :], in0=gm_half[:, 0:6, :], in1=gm_half[:, 6:12, :], op=op)
    v.tensor_tensor(out=work[:, 0:3, :], in0=work[:, 0:3, :], in1=work[:, 3:6, :], op=op)
    v.tensor_tensor(out=work[:, 0:1, :], in0=work[:, 0:1, :], in1=work[:, 2:3, :], op=op)
    v.tensor_tensor(out=out[:], in0=work[:, 0, :], in1=work[:, 1, :], op=op)


def build(cfg, repeat=1, ablate="FULL"):
    A = mybir.AluOpType
    AF = mybir.ActivationFunctionType
    NP, TILES = cfg.NP, cfg.TILES
    nc = bacc.Bacc("TRN2", target_bir_lowering=False, num_devices=CORES,
                   num_swdge_queues=NQ)

    xT = nc.dram_tensor("xT", [D, NP], F32, kind="ExternalInput")
    idx = nc.dram_tensor("idx", [TILES, 128, DEG], I32, kind="ExternalInput")
    wpack = nc.dram_tensor("wpack", [D, L, 6, D], BF16, kind="ExternalInput")
    bpack = nc.dram_tensor("bpack", [D, L], F32, kind="ExternalInput")
    outT = nc.dram_tensor("outT", [D, NP], F32, kind="ExternalOutput")

    agin = [nc.dram_tensor(f"agin{i}", [NP, 2 * D], BF16) for i in range(2)]
    agout = [nc.dram_tensor(f"agout{i}", [cfg.AG_ROWS, 2 * D], BF16, addr_space="Shared")
             for i in range(2)]

    total = repeat * L
    qstate = [0]

    with tile.TileContext(nc) as tc:
        with (
            tc.tile_pool(name="persist", bufs=1) as pp,
            tc.tile_pool(name="gat", bufs=3) as gp,
            tc.tile_pool(name="work", bufs=2) as wp,
            tc.tile_pool(name="small", bufs=3) as sp,
            tc.tile_pool(name="psum", bufs=2, space="PSUM") as ps,
        ):
            hT = [pp.tile([D, NP], F32, name=f"hT{i}") for i in range(2)]
            hB = [pp.tile([D, NP], BF16, name=f"hB{i}") for i in range(2)]
            W = pp.tile([D, L, 6, D], BF16)
            nc.sync.dma_start(out=W[:], in_=wpack[:])
            BOUT = pp.tile([D, L], F32)
            nc.sync.dma_start(out=BOUT[:], in_=bpack[:])
            IDX = pp.tile([128, TILES, DEG], I32)
            nc.sync.dma_start(out=IDX[:], in_=idx[:].rearrange("t p k -> p t k"))
            ident = pp.tile([D, D], F32)
            make_identity(nc, ident[:])
            eps = pp.tile([D, 1], F32)
            nc.vector.memset(eps[:], 144.0 * 1e-5)
            sc12 = pp.tile([D, 1], F32)
            nc.vector.memset(sc12[:], math.sqrt(12.0))
            nc.sync.dma_start(out=hT[0][:], in_=xT[:])

            def bprod(t, hb, ll_next, agin_t, agout_t):
                """B|B^2 production for the next layer from hb slab t, plus
                chunked collective when a chunk completes."""
                w2t = W[:, ll_next % L, 0, :]
                bp = ps.tile([128, 2 * D], F32, space="PSUM", tag="bp")
                nc.tensor.matmul(out=bp[:, 0:D], lhsT=hb[:, t * 128:(t + 1) * 128],
                                 rhs=w2t, start=True, stop=True)
                bn = sp.tile([128, 2 * D], BF16, tag="bn")
                nc.scalar.activation(bn[:, 0:D], bp[:, 0:D], AF.Copy)
                # table carries 12*B^2 so the var chain needs no scalar scaling
                nc.scalar.activation(bn[:, D:2 * D], bp[:, 0:D], AF.Square,
                                     scale=sc12[:])
                nc.sync.dma_start(out=agin_t[t * 128:(t + 1) * 128, :], in_=bn[:])
                for q in range(Q):
                    if t == cfg.bounds[q + 1] - 1:
                        r0, r1 = cfg.bounds[q] * 128, cfg.bounds[q + 1] * 128
                        nc.gpsimd.collective_compute(
                            "AllGather", A.bypass,
                            replica_groups=[list(range(CORES))],
                            ins=[agin_t[r0:r1, :]],
                            outs=[agout_t[CORES * r0:CORES * r1, :]],
                        )

            # ---- prologue: hB0 = bf16(x); B|B^2 for layer 0 ----
            for t in range(TILES):
                nc.scalar.activation(hB[0][:, t * 128:(t + 1) * 128],
                                     hT[0][:, t * 128:(t + 1) * 128], AF.Copy)
                bprod(t, hB[0], 0, agin[0], agout[0])

            for ll in range(total):
                l = ll % L
                last = ll == total - 1
                hin, hout = hT[ll % 2], hT[(ll + 1) % 2]
                hbin, hbout = hB[ll % 2], hB[(ll + 1) % 2]
                ag_cur = agout[ll % 2]
                whh = W[:, l, 1, :]
                weff = [W[:, l, 2 + j, :] for j in range(4)]
                bout_l = BOUT[:, l:l + 1]

                for t in range(TILES):
                    sl = slice(t * 128, (t + 1) * 128)
                    gm = gp.tile([128, DEG, 2 * D], BF16, tag="gm")
                    for k in range(DEG):
                        inst = nc.gpsimd.indirect_dma_start(
                            out=gm[:, k, :], out_offset=None, in_=ag_cur[:],
                            in_offset=bass.IndirectOffsetOnAxis(
                                ap=IDX[:, t, k:k + 1], axis=0),
                        )
                        inst.ins.queue = f"qPoolDynamic{(qstate[0] % NQ) or ''}"
                        qstate[0] += 1

                    S = sp.tile([128, D], F32, tag="S")
                    MX = sp.tile([128, D], F32, tag="MX")
                    MN = sp.tile([128, D], F32, tag="MN")
                    S2 = sp.tile([128, D], F32, tag="S2")
                    wS = wp.tile([128, 6, D], BF16, tag="wS")
                    wX = wp.tile([128, 6, D], BF16, tag="wX")
                    wN = wp.tile([128, 6, D], BF16, tag="wN")
                    w2 = wp.tile([128, 6, D], BF16, tag="w2")
                    _tree(nc, gm[:, :, 0:D], wS, S[:], "add")
                    _tree(nc, gm[:, :, 0:D], wX, MX[:], "max")
                    _tree(nc, gm[:, :, 0:D], wN, MN[:], "min")
                    _tree(nc, gm[:, :, D:2 * D], w2, S2[:], "add")

                    # var chain on Pool: varp = 144*var = S2' - S*S (S2' = 12*S2);
                    # std = sqrt(varp + 144*eps)/12 with the 1/12 folded into Ws.
                    m2 = sp.tile([128, D], F32, tag="m2")
                    nc.gpsimd.tensor_tensor(out=m2[:], in0=S[:], in1=S[:], op=A.mult)
                    varp = sp.tile([128, D], F32, tag="varp")
                    nc.gpsimd.tensor_tensor(out=varp[:], in0=S2[:], in1=m2[:],
                                            op=A.subtract)
                    varc = sp.tile([128, D], F32, tag="varc")
                    nc.gpsimd.tensor_relu(varc[:], varp[:])
                    STD = sp.tile([128, D], F32, tag="STD")
                    nc.scalar.activation(STD[:], varc[:], AF.Sqrt, bias=eps[:])

                    hp = ps.tile([128, 128], F32, space="PSUM", tag="hp")
                    nc.tensor.matmul(out=hp[:], lhsT=whh, rhs=hbin[:, sl],
                                     start=True, stop=False)
                    for j, part in enumerate([S, MX, MN, STD]):
                        ptp = ps.tile([128, 128], F32, space="PSUM", tag="tp")
                        nc.tensor.transpose(out=ptp[:], in_=part[:], identity=ident[:])
                        pbf = sp.tile([128, 128], BF16, tag="pbf")
                        nc.scalar.activation(pbf[:], ptp[:], AF.Copy)
                        nc.tensor.matmul(out=hp[:], lhsT=weff[j], rhs=pbf[:],
                                         start=False, stop=(j == 3))

                    tmpb = sp.tile([128, 128], F32, tag="tmpb")
                    nc.scalar.activation(tmpb[:], hp[:], AF.Identity, bias=bout_l)
                    nc.vector.tensor_tensor(out=hout[:, sl], in0=tmpb[:],
                                            in1=hin[:, sl], op=A.add)
                    if not last:
                        nc.scalar.activation(hbout[:, sl], hout[:, sl], AF.Copy)
                        bprod(t, hbout, ll + 1, agin[(ll + 1) % 2], agout[(ll + 1) % 2])

            nc.sync.dma_start(out=outT[:], in_=hT[total % 2][:])
    nc.compile()
    return nc


def prep_inputs(cfg, x, edge_index, Wpre, bpre, Wpost, bpost, Wlin, blin):
    x = np.asarray(x, np.float32)
    ei = np.asarray(edge_index)
    Wpre = np.asarray(Wpre, np.float32)
    bpre = np.asarray(bpre, np.float32)
    Wpost = np.asarray(Wpost, np.float32)
    bpost = np.asarray(bpost, np.float32)
    Wlin = np.asarray(Wlin, np.float32)
    blin = np.asarray(blin, np.float32)
    N, NC, NP, TILES = cfg.N, cfg.NC, cfg.NP, cfg.TILES

    row = ei[0].astype(np.int64)
    col = ei[1].astype(np.int64)
    assert (row == np.repeat(np.arange(N), DEG)).all(), "kernel assumes sorted rows, uniform degree"
    dlog = math.log(DEG + 1.0)
    k1 = dlog / AVG_LOG
    k2 = AVG_LOG / dlog

    wpack = np.zeros((D, L, 6, D), BF)
    bpack = np.zeros((D, L), np.float32)
    for l in range(L):
        W1 = Wpre[l][:, :D]
        W2 = Wpre[l][:, D:]
        WP = Wlin[l] @ Wpost[l]
        Wh = WP[:, :D]
        Weff = WP[:, D:5 * D] + k1 * WP[:, 5 * D:9 * D] + k2 * WP[:, 9 * D:13 * D]
        Wm, Wx, Wn, Ws = (Weff[:, j * D:(j + 1) * D] for j in range(4))
        Whh = Wh + (Wm + Wx + Wn) @ W1
        wpack[:, l, 0, :] = W2.T.astype(BF)
        wpack[:, l, 1, :] = Whh.T.astype(BF)
        wpack[:, l, 2, :] = (Wm / DEG).T.astype(BF)
        wpack[:, l, 3, :] = Wx.T.astype(BF)
        wpack[:, l, 4, :] = Wn.T.astype(BF)
        wpack[:, l, 5, :] = (Ws / DEG).T.astype(BF)
        bpack[:, l] = Wlin[l] @ bpost[l] + blin[l] + (Wm + Wx + Wn) @ bpre[l]

    # chunked allgather table row mapping
    b = np.array(cfg.bounds)
    rows_q = (b[1:] - b[:-1]) * 128          # rows per chunk per core
    off = CORES * b[:-1] * 128               # chunk start row in agout

    in_maps = []
    for c in range(CORES):
        xs = x[c * NC:(c + 1) * NC]
        xT = np.zeros((D, NP), np.float32)
        xT[:, :NC] = xs.T
        cols = col[c * NC * DEG:(c + 1) * NC * DEG]
        cols = np.concatenate([cols, np.zeros(((NP - NC) * DEG,), np.int64)])
        s = cols // NC
        j = cols - s * NC
        tt = j // 128
        q = cfg.chunk_of_tile[tt]
        gr = off[q] + s * rows_q[q] + (j - b[q] * 128)
        idxa = gr.reshape(TILES, 128, DEG).astype(np.int32)
        in_maps.append({
            "xT": xT,
            "idx": idxa,
            "wpack": wpack,
            "bpack": bpack,
        })
    return in_maps


_CACHE = {}


def kernel(x, edge_index, Wpre, bpre, Wpost, bpost, Wlin, blin):
    cfg = Cfg(np.asarray(x).shape[0])
    in_maps = prep_inputs(cfg, x, edge_index, Wpre, bpre, Wpost, bpost, Wlin, blin)
    if cfg.N not in _CACHE:
        _CACHE[cfg.N] = build(cfg)
    nc = _CACHE[cfg.N]
    res = run_bass_kernel_spmd(nc, in_maps, list(range(CORES)))
    outs = []
    for c in range(CORES):
        oT = res.results[c]["outT"]
        outs.append(np.ascontiguousarray(oT[:, :cfg.NC].T))
    return np.concatenate(outs, axis=0).astype(np.float32)
